# revision 13
# baseline (speedup 1.0000x reference)
"""DCT-II enhancement kernel for Trainium2 (8 NeuronCores, data parallel).

Computes out[b, n, k] = sum_d x[b, n, d] * C[k, d] where C is the 256x256
orthonormal DCT-II basis — i.e. a [B*N, 256] @ [256, 256]^T GEMM.

Sharding: pure data parallel over the flattened token dim (B*N = 131072),
16384 tokens per core. The DCT basis (transposed, [d, k]) and a 128x128
identity (for PE-transpose) are replicated to every core.

Per-core dataflow, per 512-token super-tile:
  1. DMA x tile [128p(tok), 4t, 256d] from HBM (natural layout, contiguous).
  2. PE-transpose (fp32r) the 8 [128, 128] blocks -> xT in PSUM [d, tok].
  3. Copy PSUM -> SBUF (DVE).
  4. fp32r matmuls: out[tok=128, k=256] += xT_chunk.T @ CT_chunk for the
     two 128-deep d-chunks (moving free dim 256 -> full-rate fp32r).
  5. Copy PSUM -> SBUF (DVE/ACT), DMA out to HBM in natural layout.
"""

from contextlib import ExitStack

import numpy as np

import concourse.bass as bass
import concourse.tile as tile
from concourse import bacc, mybir
from concourse.bass_utils import run_bass_kernel_spmd

P = 128
D = 256
N_CORES = 8
B, N = 32, 4096
TOK_PER_CORE = (B * N) // N_CORES  # 16384

F32 = mybir.dt.float32
F32R = mybir.dt.float32r


def dct_matrix() -> np.ndarray:
    """C[k, d] — DCT-II with ortho normalization, fp64 math cast to fp32."""
    n = D
    k = np.arange(n)[:, None].astype(np.float64)
    m = np.arange(n)[None, :].astype(np.float64)
    Cm = np.cos(np.pi * (2.0 * m + 1.0) * k / (2.0 * n))
    scale = np.full((n, 1), np.sqrt(2.0 / n))
    scale[0, 0] = np.sqrt(1.0 / n)
    return (Cm * scale).astype(np.float32)


def build_program(tok: int = TOK_PER_CORE, super_tok: int = 512,
                  num_devices: int = N_CORES) -> bass.Bass:
    """Emit the per-core Bass/Tile program. All cores run the same NEFF.

    Layout: token = i*super_tok + p*tb + s  (tb tokens per partition, so
    each partition's DMA run is tb*D*4 bytes contiguous — 4 KB at tb=4,
    512 KB per dma_start, alternating between the two HWDGE rings).

    Pipeline (3 stages, 2-iteration decoupling at every hop):
      A(i): DMA in                           (lead 3)
      B(i): 8 PE transposes -> 2 PSUM banks -> 2 SBUF copies
      C(i): 8 fp32r matmuls -> 2 PSUM banks (2 accum groups per bank)
            -> 2 SBUF copies -> DMA out
    PSUM: xt pool 4 x [128,512] banks (2/iter), out pool 4 x [128,512]
    banks (2/iter) — both 2 iterations deep. PE sees one 8-transpose
    burst then one 8-matmul burst per slot (2 mode switches).
    Copies alternate DVE/ACT by iteration parity to balance their
    measured PSUM-read rates (~1.34 vs ~2.6 ns/elem).
    """
    assert tok % super_tok == 0 and super_tok % (2 * P) == 0
    nit = tok // super_tok   # super-tile iterations
    tb = super_tok // P      # tokens per partition per super-tile
    dc = D // P              # d-chunks (contraction over 2x128)

    nc = bacc.Bacc(
        "TRN2", target_bir_lowering=False, debug=False, num_devices=num_devices
    )
    x_d = nc.dram_tensor("x", [tok, D], F32, kind="ExternalInput").ap()
    ct_d = nc.dram_tensor("ct", [D, D], F32, kind="ExternalInput").ap()
    id_d = nc.dram_tensor("ident", [P, P], F32, kind="ExternalInput").ap()
    out_d = nc.dram_tensor("out", [tok, D], F32, kind="ExternalOutput").ap()

    with ExitStack() as ctx:
        tc = ctx.enter_context(tile.TileContext(nc))
        consts = ctx.enter_context(tc.tile_pool(name="consts", bufs=1))
        xin_pool = ctx.enter_context(tc.tile_pool(name="xin", bufs=8))
        xt_sb_pool = ctx.enter_context(tc.tile_pool(name="xt_sb", bufs=4))
        out_sb_pool = ctx.enter_context(tc.tile_pool(name="out_sb", bufs=6))
        xt_ps_pool = ctx.enter_context(
            tc.tile_pool(name="xt_ps", bufs=3, space="PSUM")
        )
        out_ps_pool = ctx.enter_context(
            tc.tile_pool(name="out_ps", bufs=5, space="PSUM")
        )

        # Replicated constants: CT as [p, c, k] (d = c*128 + p), identity.
        # ident first on the sync ring (needed by the first transpose);
        # ct on the scalar ring (first needed ~10us in, keeps sync free).
        ident = consts.tile([P, P], F32R)
        nc.sync.dma_start(ident[:], id_d.bitcast(F32R))
        ct_sb = consts.tile([P, dc, D], F32R)

        def load_ct():
            nc.scalar.dma_start(
                ct_sb[:], ct_d.rearrange("(c p) k -> p c k", p=P).bitcast(F32R)
            )

        # token = i*super_tok + p*tb + s -> per-partition contiguous tb*D run
        x_t = x_d.rearrange("(i p s) d -> i p s d", p=P, s=tb)
        o_t = out_d.rearrange("(i p s) k -> i p s k", p=P, s=tb)

        rings = [nc.sync, nc.scalar]

        xins = {}
        xts = {}

        def stage_a(i):
            if not (0 <= i < nit):
                return
            if i == 0:
                # Pipeline fill: land iteration 0 as 4 per-chunk tiles with
                # precise deps so the first transpose starts ~4us earlier.
                chunks = []
                for s in range(tb):
                    xc = xin_pool.tile([P, 1, D], F32R, name=f"xin0_{s}")
                    nc.sync.dma_start(
                        xc[:], x_t[0, :, s:s + 1, :].bitcast(F32R)
                    )
                    chunks.append(xc)
                xins[i] = chunks
                return
            xin = xin_pool.tile([P, tb, D], F32R)
            nc.sync.dma_start(xin[:], x_t[i].bitcast(F32R))
            xins[i] = xin

        def copy(engine, dst, src):
            if engine == "act":
                nc.scalar.copy(dst, src)
            else:
                nc.vector.tensor_copy(dst, src)

        def stage_b(i):
            """Transposes (one 8-burst) + xT PSUM->SBUF copies."""
            if not (0 <= i < nit):
                return
            xin = xins.pop(i)

            def xin_slice(s, c):
                if isinstance(xin, list):
                    return xin[s][:, 0, c * P:(c + 1) * P]
                return xin[:, s, c * P:(c + 1) * P]

            xt_sb = xt_sb_pool.tile([P, dc, super_tok], F32R)
            xts[i] = xt_sb
            xt_pss = []
            for c in range(dc):
                xt_ps = xt_ps_pool.tile([P, super_tok], F32R)
                xt_pss.append(xt_ps)
                for s in range(tb):
                    nc.tensor.transpose(
                        xt_ps[:, s * P:(s + 1) * P],
                        xin_slice(s, c),
                        ident[:],
                    )
            # xT copies gate the matmuls -> always on the fast DVE.
            copy("dve", xt_sb[:, 0, :], xt_pss[0][:])
            copy("dve", xt_sb[:, 1, :], xt_pss[1][:])

        def stage_c(i):
            """Matmuls (one 8-burst into 2 banks) + out copies + DMA out."""
            if not (0 <= i < nit):
                return
            xt_sb = xts.pop(i)
            out_sb = out_sb_pool.tile([P, tb, D], F32)
            out_pss = []
            for sp in range(tb // 2):
                out_ps = out_ps_pool.tile([P, 2 * D], F32)
                out_pss.append(out_ps)
                for s_in in range(2):
                    s = 2 * sp + s_in
                    for c in range(dc):
                        nc.tensor.matmul(
                            out_ps[:, s_in * D:(s_in + 1) * D],
                            xt_sb[:, c, s * P:(s + 1) * P],
                            ct_sb[:, c, :],
                            start=(c == 0),
                            stop=(c == dc - 1),
                        )
            # Out copies have ~2 iterations of slack (PSUM depth + out_sb
            # bufs): balance DVE/ACT by alternating the first copy's engine.
            eng0 = "act" if i % 2 == 0 else "dve"
            copy(eng0, out_sb[:, 0:2, :], out_pss[0][:])
            if i >= nit - 2:
                # Drain the tail sooner: ship each half as soon as copied.
                nc.scalar.dma_start(o_t[i, :, 0:2, :], out_sb[:, 0:2, :])
                copy("act", out_sb[:, 2:4, :], out_pss[1][:])
                nc.scalar.dma_start(o_t[i, :, 2:4, :], out_sb[:, 2:4, :])
            else:
                copy("act", out_sb[:, 2:4, :], out_pss[1][:])
                nc.scalar.dma_start(o_t[i], out_sb[:])

        stage_a(0)
        load_ct()
        stage_a(1)
        stage_a(2)
        for i in range(nit + 1):
            stage_a(i + 3)
            stage_b(i)
            stage_c(i - 1)

    nc.compile()
    return nc


_PROGRAM_CACHE: dict = {}


def _get_program() -> bass.Bass:
    if "nc" not in _PROGRAM_CACHE:
        _PROGRAM_CACHE["nc"] = build_program()
    return _PROGRAM_CACHE["nc"]


def make_in_maps(x_flat: np.ndarray) -> list[dict]:
    ct = np.ascontiguousarray(dct_matrix().T)  # [d, k]
    ident = np.eye(P, dtype=np.float32)
    shards = x_flat.reshape(N_CORES, TOK_PER_CORE, D)
    return [
        {"x": np.ascontiguousarray(shards[i]), "ct": ct, "ident": ident}
        for i in range(N_CORES)
    ]


def kernel(x: np.ndarray) -> np.ndarray:
    x = np.ascontiguousarray(np.asarray(x, dtype=np.float32))
    b, n, d = x.shape
    assert (b, n, d) == (B, N, D), f"unexpected shape {x.shape}"
    nc = _get_program()
    in_maps = make_in_maps(x.reshape(b * n, d))
    res = run_bass_kernel_spmd(nc, in_maps, core_ids=list(range(N_CORES)))
    out = np.concatenate([r["out"] for r in res.results], axis=0)
    return out.reshape(b, n, d)


# revision 14
# speedup vs baseline: 1.0302x; 1.0302x over previous
"""DCT-II enhancement kernel for Trainium2 (8 NeuronCores, data parallel).

Computes out[b, n, k] = sum_d x[b, n, d] * C[k, d] where C is the 256x256
orthonormal DCT-II basis — i.e. a [B*N, 256] @ [256, 256]^T GEMM.

Sharding: pure data parallel over the flattened token dim (B*N = 131072),
16384 tokens per core. The DCT basis (transposed, [d, k]) and a 128x128
identity (for PE-transpose) are replicated to every core.

Per-core dataflow, per 512-token super-tile:
  1. DMA x tile [128p(tok), 4t, 256d] from HBM (natural layout, contiguous).
  2. PE-transpose (fp32r) the 8 [128, 128] blocks -> xT in PSUM [d, tok].
  3. Copy PSUM -> SBUF (DVE).
  4. fp32r matmuls: out[tok=128, k=256] += xT_chunk.T @ CT_chunk for the
     two 128-deep d-chunks (moving free dim 256 -> full-rate fp32r).
  5. Copy PSUM -> SBUF (DVE/ACT), DMA out to HBM in natural layout.
"""

from contextlib import ExitStack

import numpy as np

import concourse.bass as bass
import concourse.tile as tile
from concourse import bacc, mybir
from concourse.bass_utils import run_bass_kernel_spmd

P = 128
D = 256
N_CORES = 8
B, N = 32, 4096
TOK_PER_CORE = (B * N) // N_CORES  # 16384

F32 = mybir.dt.float32
F32R = mybir.dt.float32r


def dct_matrix() -> np.ndarray:
    """C[k, d] — DCT-II with ortho normalization, fp64 math cast to fp32."""
    n = D
    k = np.arange(n)[:, None].astype(np.float64)
    m = np.arange(n)[None, :].astype(np.float64)
    Cm = np.cos(np.pi * (2.0 * m + 1.0) * k / (2.0 * n))
    scale = np.full((n, 1), np.sqrt(2.0 / n))
    scale[0, 0] = np.sqrt(1.0 / n)
    return (Cm * scale).astype(np.float32)


def build_program(tok: int = TOK_PER_CORE, super_tok: int = 512,
                  num_devices: int = N_CORES) -> bass.Bass:
    """Emit the per-core Bass/Tile program. All cores run the same NEFF.

    Layout: token = i*super_tok + p*tb + s  (tb tokens per partition, so
    each partition's DMA run is tb*D*4 bytes contiguous — 4 KB at tb=4,
    512 KB per dma_start, alternating between the two HWDGE rings).

    Pipeline (3 stages, 2-iteration decoupling at every hop):
      A(i): DMA in                           (lead 3)
      B(i): 8 PE transposes -> 2 PSUM banks -> 2 SBUF copies
      C(i): 8 fp32r matmuls -> 2 PSUM banks (2 accum groups per bank)
            -> 2 SBUF copies -> DMA out
    PSUM: xt pool 4 x [128,512] banks (2/iter), out pool 4 x [128,512]
    banks (2/iter) — both 2 iterations deep. PE sees one 8-transpose
    burst then one 8-matmul burst per slot (2 mode switches).
    Copies alternate DVE/ACT by iteration parity to balance their
    measured PSUM-read rates (~1.34 vs ~2.6 ns/elem).
    """
    assert tok % super_tok == 0 and super_tok % (2 * P) == 0
    nit = tok // super_tok   # super-tile iterations
    tb = super_tok // P      # tokens per partition per super-tile
    dc = D // P              # d-chunks (contraction over 2x128)

    nc = bacc.Bacc(
        "TRN2", target_bir_lowering=False, debug=False, num_devices=num_devices
    )
    x_d = nc.dram_tensor("x", [tok, D], F32, kind="ExternalInput").ap()
    ct_d = nc.dram_tensor("ct", [D, D], F32, kind="ExternalInput").ap()
    id_d = nc.dram_tensor("ident", [P, P], F32, kind="ExternalInput").ap()
    out_d = nc.dram_tensor("out", [tok, D], F32, kind="ExternalOutput").ap()

    with ExitStack() as ctx:
        tc = ctx.enter_context(tile.TileContext(nc))
        consts = ctx.enter_context(tc.tile_pool(name="consts", bufs=1))
        xin_pool = ctx.enter_context(tc.tile_pool(name="xin", bufs=10))
        xt_sb_pool = ctx.enter_context(tc.tile_pool(name="xt_sb", bufs=5))
        out_sb_pool = ctx.enter_context(tc.tile_pool(name="out_sb", bufs=6))
        xt_ps_pool = ctx.enter_context(
            tc.tile_pool(name="xt_ps", bufs=3, space="PSUM")
        )
        out_ps_pool = ctx.enter_context(
            tc.tile_pool(name="out_ps", bufs=5, space="PSUM")
        )

        # Replicated constants: CT as [p, c, k] (d = c*128 + p), identity.
        # ident first on the sync ring (needed by the first transpose);
        # ct on the scalar ring (first needed ~10us in, keeps sync free).
        ident = consts.tile([P, P], F32R)
        nc.sync.dma_start(ident[:], id_d.bitcast(F32R))
        ct_sb = consts.tile([P, dc, D], F32R)

        def load_ct():
            nc.scalar.dma_start(
                ct_sb[:], ct_d.rearrange("(c p) k -> p c k", p=P).bitcast(F32R)
            )

        # token = i*super_tok + p*tb + s -> per-partition contiguous tb*D run
        x_t = x_d.rearrange("(i p s) d -> i p s d", p=P, s=tb)
        o_t = out_d.rearrange("(i p s) k -> i p s k", p=P, s=tb)

        rings = [nc.sync, nc.scalar]

        xins = {}
        xts = {}

        def stage_a(i):
            if not (0 <= i < nit):
                return
            if i == 0:
                # Pipeline fill: land iteration 0 as 4 per-chunk tiles with
                # precise deps so the first transpose starts ~4us earlier.
                chunks = []
                for s in range(tb):
                    xc = xin_pool.tile([P, 1, D], F32R, name=f"xin0_{s}",
                                       bufs=1)
                    rings[s % 2].dma_start(
                        xc[:], x_t[0, :, s:s + 1, :].bitcast(F32R)
                    )
                    chunks.append(xc)
                xins[i] = chunks
                return
            xin = xin_pool.tile([P, tb, D], F32R)
            nc.sync.dma_start(xin[:], x_t[i].bitcast(F32R))
            xins[i] = xin

        def copy(engine, dst, src):
            if engine == "act":
                nc.scalar.copy(dst, src)
            else:
                nc.vector.tensor_copy(dst, src)

        def stage_b(i):
            """Transposes (one 8-burst) + xT PSUM->SBUF copies."""
            if not (0 <= i < nit):
                return
            xin = xins.pop(i)

            def xin_slice(s, c):
                if isinstance(xin, list):
                    return xin[s][:, 0, c * P:(c + 1) * P]
                return xin[:, s, c * P:(c + 1) * P]

            xt_sb = xt_sb_pool.tile([P, dc, super_tok], F32R)
            xts[i] = xt_sb
            xt_pss = []
            for c in range(dc):
                xt_ps = xt_ps_pool.tile([P, super_tok], F32R)
                xt_pss.append(xt_ps)
                for s in range(tb):
                    nc.tensor.transpose(
                        xt_ps[:, s * P:(s + 1) * P],
                        xin_slice(s, c),
                        ident[:],
                    )
            # xT copies gate the matmuls -> always on the fast DVE.
            copy("dve", xt_sb[:, 0, :], xt_pss[0][:])
            copy("dve", xt_sb[:, 1, :], xt_pss[1][:])

        def stage_c(i):
            """Matmuls (one 8-burst into 2 banks) + out copies + DMA out."""
            if not (0 <= i < nit):
                return
            xt_sb = xts.pop(i)
            out_sb = out_sb_pool.tile([P, tb, D], F32)
            out_pss = []
            for sp in range(tb // 2):
                out_ps = out_ps_pool.tile([P, 2 * D], F32)
                out_pss.append(out_ps)
                for s_in in range(2):
                    s = 2 * sp + s_in
                    for c in range(dc):
                        nc.tensor.matmul(
                            out_ps[:, s_in * D:(s_in + 1) * D],
                            xt_sb[:, c, s * P:(s + 1) * P],
                            ct_sb[:, c, :],
                            start=(c == 0),
                            stop=(c == dc - 1),
                        )
            # Out copies have ~2 iterations of slack (PSUM depth + out_sb
            # bufs): balance DVE/ACT by alternating the first copy's engine.
            eng0 = "act" if i % 2 == 0 else "dve"
            copy(eng0, out_sb[:, 0:2, :], out_pss[0][:])
            if i >= nit - 2:
                # Drain the tail sooner: ship each half as soon as copied.
                nc.scalar.dma_start(o_t[i, :, 0:2, :], out_sb[:, 0:2, :])
                copy("act", out_sb[:, 2:4, :], out_pss[1][:])
                nc.scalar.dma_start(o_t[i, :, 2:4, :], out_sb[:, 2:4, :])
            else:
                copy("act", out_sb[:, 2:4, :], out_pss[1][:])
                nc.scalar.dma_start(o_t[i], out_sb[:])

        stage_a(0)
        load_ct()
        stage_a(1)
        stage_a(2)
        for i in range(nit + 1):
            stage_a(i + 3)
            stage_b(i)
            stage_c(i - 1)

    nc.compile()
    return nc


_PROGRAM_CACHE: dict = {}


def _get_program() -> bass.Bass:
    if "nc" not in _PROGRAM_CACHE:
        _PROGRAM_CACHE["nc"] = build_program()
    return _PROGRAM_CACHE["nc"]


def make_in_maps(x_flat: np.ndarray) -> list[dict]:
    ct = np.ascontiguousarray(dct_matrix().T)  # [d, k]
    ident = np.eye(P, dtype=np.float32)
    shards = x_flat.reshape(N_CORES, TOK_PER_CORE, D)
    return [
        {"x": np.ascontiguousarray(shards[i]), "ct": ct, "ident": ident}
        for i in range(N_CORES)
    ]


def kernel(x: np.ndarray) -> np.ndarray:
    x = np.ascontiguousarray(np.asarray(x, dtype=np.float32))
    b, n, d = x.shape
    assert (b, n, d) == (B, N, D), f"unexpected shape {x.shape}"
    nc = _get_program()
    in_maps = make_in_maps(x.reshape(b * n, d))
    res = run_bass_kernel_spmd(nc, in_maps, core_ids=list(range(N_CORES)))
    out = np.concatenate([r["out"] for r in res.results], axis=0)
    return out.reshape(b, n, d)
